# revision 4
# baseline (speedup 1.0000x reference)
# Trainium2 Bass kernel for nn_AttentionRecognitionHead (attention decoder:
# 25-step GRU with additive attention over T=1024 encoder positions).
#
# Sharding: data-parallel over batch. Each of the 8 cores gets B_LOC=32 rows
# of x/targets; the small decoder parameters are replicated. No collectives.
#
# Per-core algorithm (all shapes per core):
#   precompute (device):
#     xh   = fp16(x)                                  [32,1024,512] DRAM
#     xpjT = fp16(x @ xW.T + xb), stored [a,t]        [32,4,128,1024] DRAM
#   per step i (25x, sequential):
#     tanh tiles  [128(a),1024(t)] = tanh(xpjT + sProjT[:,ac,b])   (ACT, bias)
#     scores[b,t] = sum_a wW[a]*tanh  (PE: lhsT=wW[128,1], accum over 4 a-chunks)
#     softmax over t (no max-sub needed: |scores|<=~4; wb dropped: softmax-shift
#     invariant), alpha -> alphaT [128(t),8,32] via PE transpose
#     context[b,d] = sum_t alpha*x  (PE: lhsT=alphaT[:,tk,b], rhs=xh tiles)
#     GRU (f32): gi = [yp;ctx] @ W_ih.T, gh = h @ W_hh.T, gates, h update
#     out[:,i,:] = h_new @ fcW.T + fcb
#
# fp16 is used for the two large per-step streams (xpjT, xh) and the tiny
# attention operands (wW, alphaT); everything else stays f32.

import os
import numpy as np

import concourse.bass as bass
import concourse.tile as tile
import concourse.mybir as mybir
from concourse import bacc
from concourse.masks import make_identity

F32 = mybir.dt.float32
F16 = mybir.dt.float16
AF = mybir.ActivationFunctionType
OP = mybir.AluOpType

B, T, DX = 256, 1024, 512
S, A = 512, 512
C, L = 97, 25
NCORES = 8
BL = B // NCORES  # 32


def build(bl=BL, td=T, steps=L, sc_bufs=3, stream_bufs=6):
    """Build the per-core SPMD program. Returns a finalized Bacc."""
    nc = bacc.Bacc(trn_type="TRN2")
    TQ = td // 512   # 512-wide t chunks
    TK = td // 128   # 128-wide t chunks
    PP = td // 256   # 256-wide t chunks (x-stream pair tiles)

    x_in = nc.dram_tensor("xin", [bl, td, DX], F32, kind="ExternalInput")
    xWT_d = nc.dram_tensor("xWT", [DX, A], F16, kind="ExternalInput")
    wW_d = nc.dram_tensor("wWh", [A], F16, kind="ExternalInput")
    sWT_d = nc.dram_tensor("sWT", [S, A], F16, kind="ExternalInput")
    WihT_d = nc.dram_tensor("WihT", [A + DX, 3 * S], F16, kind="ExternalInput")
    WhhT_d = nc.dram_tensor("WhhT", [S, 3 * S], F16, kind="ExternalInput")
    fcWT_d = nc.dram_tensor("fcWT", [S, C], F16, kind="ExternalInput")
    ypT_d = nc.dram_tensor("ypT", [steps, A, bl], F16, kind="ExternalInput")
    sbv_d = nc.dram_tensor("sbv", [A], F32, kind="ExternalInput")
    xbv_d = nc.dram_tensor("xbv", [A], F32, kind="ExternalInput")
    bih_d = nc.dram_tensor("bihv", [3 * S], F32, kind="ExternalInput")
    bhh_d = nc.dram_tensor("bhhv", [3 * S], F32, kind="ExternalInput")
    fcb_d = nc.dram_tensor("fcbv", [C], F32, kind="ExternalInput")
    out_d = nc.dram_tensor("out", [bl, steps, C], F32, kind="ExternalOutput")

    def bcast_rows(ap, p):
        # replicate a free-dim-only AP across p partitions (step-0 partition dim)
        return bass.AP(tensor=ap.tensor, offset=ap.offset, ap=[[0, p]] + list(ap.ap))

    with tile.TileContext(nc) as tc:
        with tc.tile_pool(name="const", bufs=1) as CP, \
             tc.tile_pool(name="state", bufs=1) as SP, \
             tc.tile_pool(name="dram", bufs=1, space="DRAM") as DP:
            # ---- replicated parameters to SBUF ----
            xWT_sb = CP.tile([128, 4, A], F16)
            nc.sync.dma_start(xWT_sb, xWT_d[:].rearrange("(dc p) a -> p dc a", p=128))
            wW_sb = CP.tile([128, 4], F16)
            nc.sync.dma_start(wW_sb, wW_d[:].rearrange("(ac p) -> p ac", p=128))
            sWT_sb = CP.tile([128, 4, A], F16)
            nc.sync.dma_start(sWT_sb, sWT_d[:].rearrange("(sc p) a -> p sc a", p=128))
            WihT_sb = CP.tile([128, 8, 3 * S], F16)
            nc.sync.dma_start(WihT_sb, WihT_d[:].rearrange("(ic p) o -> p ic o", p=128))
            WhhT_sb = CP.tile([128, 4, 3 * S], F16)
            nc.sync.dma_start(WhhT_sb, WhhT_d[:].rearrange("(sc p) o -> p sc o", p=128))
            fcWT_sb = CP.tile([128, 4, C], F16)
            nc.sync.dma_start(fcWT_sb, fcWT_d[:].rearrange("(sc p) c -> p sc c", p=128))
            ypT_sb = CP.tile([128, steps, 4, bl], F16)
            nc.sync.dma_start(ypT_sb, ypT_d[:].rearrange("l (ac p) b -> p l ac b", p=128))
            sbv_sb = CP.tile([128, 4], F32)
            nc.sync.dma_start(sbv_sb, sbv_d[:].rearrange("(ac p) -> p ac", p=128))
            xbv_sb = CP.tile([128, 4], F32)
            nc.sync.dma_start(xbv_sb, xbv_d[:].rearrange("(ac p) -> p ac", p=128))
            bihrep = CP.tile([bl, 3 * S], F32)
            nc.gpsimd.dma_start(out=bihrep, in_=bcast_rows(bih_d[:], bl))
            bhhrep = CP.tile([bl, 3 * S], F32)
            nc.gpsimd.dma_start(out=bhhrep, in_=bcast_rows(bhh_d[:], bl))
            fcbrep = CP.tile([bl, C], F32)
            nc.gpsimd.dma_start(out=fcbrep, in_=bcast_rows(fcb_d[:], bl))
            ident = CP.tile([32, 32], F32)
            make_identity(nc, ident)

            # ---- persistent state ----
            h_sb = SP.tile([bl, S], F32)
            hT_sb = SP.tile([128, 4, bl], F16)
            sPT_sb = SP.tile([128, 4, bl], F32)
            scores_sb = SP.tile([bl, td], F32)
            alpha_sb = SP.tile([bl, td], F32)
            ssum_sb = SP.tile([bl, 1], F32)
            rinv_sb = SP.tile([bl, 1], F32)
            alphaT_sb = SP.tile([128, TK, bl], F16)
            ctx_sb = SP.tile([bl, DX], F32)
            ctxT_sb = SP.tile([128, 4, bl], F16)
            u_sb = SP.tile([bl, 3 * S], F32)
            v_sb = SP.tile([bl, 3 * S], F32)
            r_sb = SP.tile([bl, S], F32)
            z_sb = SP.tile([bl, S], F32)
            n_sb = SP.tile([bl, S], F32)
            ta_sb = SP.tile([bl, S], F32)
            tb_sb = SP.tile([bl, S], F32)
            tg_sb = SP.tile([bl, S], F32)
            outbuf = SP.tile([bl, steps, C], F32)

            xh_d = DP.tile([bl, td, DX], F16)
            xpj_d = DP.tile([bl, 4, 128, td], F16)

            nc.vector.memset(h_sb, 0.0)
            nc.vector.memset(hT_sb, 0.0)
            # step-0 sProjT = sb (h0 = 0), broadcast across b
            nc.vector.memset(sPT_sb, 0.0)
            for ac in range(4):
                nc.vector.tensor_scalar_add(out=sPT_sb[:, ac, :],
                                            in0=sPT_sb[:, ac, :],
                                            scalar1=sbv_sb[:, ac:ac + 1])

            def tr(ps, in_ap):
                p = in_ap.shape[0]
                nc.tensor.transpose(ps, in_ap, ident[:p, :p])

            # ---- precompute: cast x to fp16; xpjT = fp16(x @ xW.T + xb) ----
            with tc.tile_pool(name="pre", bufs=2) as PRE, \
                 tc.tile_pool(name="prep", bufs=4, space="PSUM") as PPS, \
                 tc.tile_pool(name="xt", bufs=6) as XT:
                xr = x_in[:].rearrange("b (tq j p) d -> b tq j p d", j=4, p=128)
                xh_w = xh_d[:].rearrange("b (tq j p) d -> b tq p j d", j=4, p=128)
                for b in range(bl):
                    for tq in range(TQ):
                        cin = PRE.tile([128, 4, DX], F32, tag="cin")
                        nc.sync.dma_start(cin, xr[b, tq].rearrange("j p d -> p j d"))
                        cot = PRE.tile([128, 4, DX], F16, tag="cot")
                        nc.vector.tensor_copy(out=cot, in_=cin)
                        nc.sync.dma_start(xh_w[b, tq], cot)
                for b in range(bl):
                    xts = []
                    for dc in range(4):
                        xt_t = XT.tile([128, td], F16, tag="xt")
                        nc.sync.dma_start_transpose(xt_t, xh_d[b, :, dc * 128:(dc + 1) * 128])
                        xts.append(xt_t)
                    for ac in range(4):
                        for tcc in range(TQ):
                            ps = PPS.tile([128, 512], F32, tag="pps")
                            for dc in range(4):
                                nc.tensor.matmul(
                                    ps,
                                    xWT_sb[:, dc, ac * 128:(ac + 1) * 128],
                                    xts[dc][:, tcc * 512:(tcc + 1) * 512],
                                    start=(dc == 0), stop=(dc == 3),
                                )
                            po = PRE.tile([128, 512], F16, tag="po")
                            nc.scalar.activation(out=po, in_=ps, func=AF.Identity,
                                                 bias=xbv_sb[:, ac:ac + 1])
                            nc.sync.dma_start(xpj_d[b, ac, :, tcc * 512:(tcc + 1) * 512], po)

            # ---- main sequential loop ----
            with tc.tile_pool(name="xp", bufs=stream_bufs) as XP, \
                 tc.tile_pool(name="th", bufs=4) as TH, \
                 tc.tile_pool(name="xs", bufs=stream_bufs) as XS, \
                 tc.tile_pool(name="row", bufs=4) as ROW, \
                 tc.tile_pool(name="psrow", bufs=sc_bufs, space="PSUM") as PS_ROW, \
                 tc.tile_pool(name="pstr", bufs=2, space="PSUM") as PS_TR, \
                 tc.tile_pool(name="psg", bufs=2, space="PSUM") as PS_G, \
                 tc.tile_pool(name="psfc", bufs=1, space="PSUM") as PS_FC:
                xs_r = xh_d[:].rearrange("b (pp j p) d -> b pp p j d", j=2, p=128)
                for i in range(steps):
                    # -- scores --
                    for b in range(bl):
                        srow = ROW.tile([1, td], F32, tag="srow")
                        scps = [PS_ROW.tile([1, 512], F32, tag="row", name=f"scps{q}")
                                for q in range(TQ)]
                        for ac in range(4):
                            xpt = XP.tile([128, td], F16, tag="xp")
                            nc.sync.dma_start(xpt, xpj_d[b, ac])
                            tht = TH.tile([128, td], F16, tag="th")
                            nc.scalar.activation(out=tht, in_=xpt, func=AF.Tanh,
                                                 bias=sPT_sb[:, ac, b:b + 1])
                            for tcc in range(TQ):
                                nc.tensor.matmul(
                                    scps[tcc], wW_sb[:, ac:ac + 1],
                                    tht[:, tcc * 512:(tcc + 1) * 512],
                                    start=(ac == 0), stop=(ac == 3),
                                )
                        for tcc in range(TQ):
                            nc.vector.tensor_copy(out=srow[:, tcc * 512:(tcc + 1) * 512],
                                                  in_=scps[tcc])
                        nc.sync.dma_start(scores_sb[b:b + 1, :], srow)
                    # -- softmax + alphaT --
                    nc.scalar.activation(out=alpha_sb, in_=scores_sb, func=AF.Exp,
                                         accum_out=ssum_sb)
                    nc.vector.reciprocal(rinv_sb, ssum_sb)
                    nc.vector.tensor_scalar_mul(alpha_sb, alpha_sb, rinv_sb)
                    for tk in range(TK):
                        tp = PS_TR.tile([128, bl], F32, tag="tr")
                        tr(tp, alpha_sb[:, tk * 128:(tk + 1) * 128])
                        nc.vector.tensor_copy(out=alphaT_sb[:, tk, :], in_=tp)
                    # -- context --
                    for b in range(bl):
                        cps = PS_ROW.tile([1, DX], F32, tag="row")
                        for pp in range(PP):
                            xst = XS.tile([128, 2, DX], F16, tag="xs")
                            nc.sync.dma_start(xst, xs_r[b, pp])
                            for j in range(2):
                                tk = pp * 2 + j
                                nc.tensor.matmul(cps, alphaT_sb[:, tk, b:b + 1],
                                                 xst[:, j, :],
                                                 start=(tk == 0), stop=(tk == TK - 1))
                        crow = ROW.tile([1, DX], F32, tag="crow")
                        nc.vector.tensor_copy(out=crow, in_=cps)
                        nc.sync.dma_start(ctx_sb[b:b + 1, :], crow)
                    for sc in range(4):
                        tp = PS_TR.tile([128, bl], F32, tag="tr")
                        tr(tp, ctx_sb[:, sc * 128:(sc + 1) * 128])
                        nc.vector.tensor_copy(out=ctxT_sb[:, sc, :], in_=tp)
                    # -- GRU --
                    for oc in range(3):
                        gip = PS_G.tile([bl, 512], F32, tag="g")
                        for ic in range(8):
                            lhsT = ypT_sb[:, i, ic, :] if ic < 4 else ctxT_sb[:, ic - 4, :]
                            nc.tensor.matmul(gip, lhsT,
                                             WihT_sb[:, ic, oc * 512:(oc + 1) * 512],
                                             start=(ic == 0), stop=(ic == 7))
                        ghp = PS_G.tile([bl, 512], F32, tag="g")
                        for sc in range(4):
                            nc.tensor.matmul(ghp, hT_sb[:, sc, :],
                                             WhhT_sb[:, sc, oc * 512:(oc + 1) * 512],
                                             start=(sc == 0), stop=(sc == 3))
                        nc.vector.tensor_tensor(v_sb[:, oc * 512:(oc + 1) * 512], gip,
                                                bihrep[:, oc * 512:(oc + 1) * 512], OP.add)
                        nc.vector.tensor_tensor(u_sb[:, oc * 512:(oc + 1) * 512], ghp,
                                                bhhrep[:, oc * 512:(oc + 1) * 512], OP.add)
                    nc.vector.tensor_tensor(ta_sb, v_sb[:, 0:512], u_sb[:, 0:512], OP.add)
                    nc.scalar.activation(out=r_sb, in_=ta_sb, func=AF.Sigmoid)
                    nc.vector.tensor_tensor(tb_sb, v_sb[:, 512:1024], u_sb[:, 512:1024], OP.add)
                    nc.scalar.activation(out=z_sb, in_=tb_sb, func=AF.Sigmoid)
                    nc.vector.tensor_tensor(tg_sb, r_sb, u_sb[:, 1024:1536], OP.mult)
                    nc.vector.tensor_tensor(tg_sb, v_sb[:, 1024:1536], tg_sb, OP.add)
                    nc.scalar.activation(out=n_sb, in_=tg_sb, func=AF.Tanh)
                    nc.vector.tensor_tensor(ta_sb, h_sb, n_sb, OP.subtract)
                    nc.vector.tensor_tensor(tb_sb, z_sb, ta_sb, OP.mult)
                    nc.vector.tensor_tensor(h_sb, n_sb, tb_sb, OP.add)
                    for sc in range(4):
                        tp = PS_TR.tile([128, bl], F32, tag="tr")
                        tr(tp, h_sb[:, sc * 128:(sc + 1) * 128])
                        nc.vector.tensor_copy(out=hT_sb[:, sc, :], in_=tp)
                    if i + 1 < steps:
                        for ac in range(4):
                            sp = PS_TR.tile([128, bl], F32, tag="tr")
                            for sc in range(4):
                                nc.tensor.matmul(sp,
                                                 sWT_sb[:, sc, ac * 128:(ac + 1) * 128],
                                                 hT_sb[:, sc, :],
                                                 start=(sc == 0), stop=(sc == 3))
                            nc.scalar.activation(out=sPT_sb[:, ac, :], in_=sp,
                                                 func=AF.Identity,
                                                 bias=sbv_sb[:, ac:ac + 1])
                    fp = PS_FC.tile([bl, C], F32, tag="fc")
                    for sc in range(4):
                        nc.tensor.matmul(fp, hT_sb[:, sc, :], fcWT_sb[:, sc, :],
                                         start=(sc == 0), stop=(sc == 3))
                    nc.vector.tensor_tensor(outbuf[:, i, :], fp, fcbrep, OP.add)
                nc.sync.dma_start(out_d[:], outbuf[:])

    nc.finalize()
    return nc


def host_prep(inputs, bl=BL, td=T, steps=L, ncores=NCORES):
    """Split the full inputs into per-core in_maps."""
    x = np.ascontiguousarray(np.asarray(inputs["x"], dtype=np.float32))
    targets = np.asarray(inputs["targets"], dtype=np.int32)
    emb = np.asarray(inputs["emb"], dtype=np.float32)
    num_classes = np.asarray(inputs["fcW"]).shape[0]
    nb = bl * ncores
    y_prev = np.concatenate(
        [np.full((nb, 1), num_classes, np.int32), targets[:nb, :steps - 1]], axis=1)
    yp = emb[y_prev]  # [nb, steps, A]

    shared = {
        "xWT": np.ascontiguousarray(np.asarray(inputs["xW"], np.float32).T).astype(np.float16),
        "wWh": np.asarray(inputs["wW"], np.float32).astype(np.float16),
        "sWT": np.ascontiguousarray(np.asarray(inputs["sW"], np.float32).T).astype(np.float16),
        "WihT": np.ascontiguousarray(np.asarray(inputs["W_ih"], np.float32).T).astype(np.float16),
        "WhhT": np.ascontiguousarray(np.asarray(inputs["W_hh"], np.float32).T).astype(np.float16),
        "fcWT": np.ascontiguousarray(np.asarray(inputs["fcW"], np.float32).T).astype(np.float16),
        "sbv": np.asarray(inputs["sb"], np.float32),
        "xbv": np.asarray(inputs["xb"], np.float32),
        "bihv": np.asarray(inputs["b_ih"], np.float32),
        "bhhv": np.asarray(inputs["b_hh"], np.float32),
        "fcbv": np.asarray(inputs["fcb"], np.float32),
    }
    in_maps = []
    for c in range(ncores):
        sl = slice(c * bl, (c + 1) * bl)
        m = dict(shared)
        m["xin"] = np.ascontiguousarray(x[sl, :td, :])
        m["ypT"] = np.ascontiguousarray(np.transpose(yp[sl], (1, 2, 0))).astype(np.float16)
        in_maps.append(m)
    return in_maps


_CACHE = {}
LAST_RESULTS = None


def kernel(**inputs):
    global LAST_RESULTS
    from concourse.bass_utils import run_bass_kernel_spmd
    if "nc" not in _CACHE:
        _CACHE["nc"] = build()
    nc = _CACHE["nc"]
    in_maps = host_prep(inputs)
    trace = bool(os.environ.get("ATH_TRACE"))
    res = run_bass_kernel_spmd(nc, in_maps, core_ids=list(range(NCORES)), trace=trace)
    LAST_RESULTS = res
    out = np.concatenate([res.results[c]["out"] for c in range(NCORES)], axis=0)
    return np.ascontiguousarray(out.astype(np.float32))


# revision 6
# speedup vs baseline: 1.0332x; 1.0332x over previous
# Trainium2 Bass kernel for nn_AttentionRecognitionHead (attention decoder:
# 25-step GRU with additive attention over T=1024 encoder positions).
#
# Sharding: data-parallel over batch. Each of the 8 cores gets B_LOC=32 rows
# of x/targets; the small decoder parameters are replicated. No collectives.
#
# Per-core algorithm (all shapes per core):
#   precompute (device):
#     xh   = fp16(x)                                  [32,1024,512] DRAM
#     xpjT = fp16(x @ xW.T + xb), stored [a,t]        [32,4,128,1024] DRAM
#   per step i (25x, sequential):
#     tanh tiles  [128(a),1024(t)] = tanh(xpjT + sProjT[:,ac,b])   (ACT, bias)
#     scores[b,t] = sum_a wW[a]*tanh  (PE: lhsT=wW[128,1], accum over 4 a-chunks)
#     softmax over t (no max-sub needed: |scores|<=~4; wb dropped: softmax-shift
#     invariant), alpha -> alphaT [128(t),8,32] via PE transpose
#     context[b,d] = sum_t alpha*x  (PE: lhsT=alphaT[:,tk,b], rhs=xh tiles)
#     GRU (f32): gi = [yp;ctx] @ W_ih.T, gh = h @ W_hh.T, gates, h update
#     out[:,i,:] = h_new @ fcW.T + fcb
#
# fp16 is used for the two large per-step streams (xpjT, xh) and the tiny
# attention operands (wW, alphaT); everything else stays f32.

import os
import numpy as np

import concourse.bass as bass
import concourse.tile as tile
import concourse.mybir as mybir
from concourse import bacc
from concourse.masks import make_identity

F32 = mybir.dt.float32
F16 = mybir.dt.float16
AF = mybir.ActivationFunctionType
OP = mybir.AluOpType

B, T, DX = 256, 1024, 512
S, A = 512, 512
C, L = 97, 25
NCORES = 8
BL = B // NCORES  # 32


def build(bl=BL, td=T, steps=L, sc_bufs=3, stream_bufs=6):
    """Build the per-core SPMD program. Returns a finalized Bacc."""
    nc = bacc.Bacc(trn_type="TRN2")
    TQ = td // 512   # 512-wide t chunks
    TK = td // 128   # 128-wide t chunks
    PP = td // 256   # 256-wide t chunks (x-stream pair tiles)

    x_in = nc.dram_tensor("xin", [bl, td, DX], F32, kind="ExternalInput")
    xWT_d = nc.dram_tensor("xWT", [DX, A], F16, kind="ExternalInput")
    wW_d = nc.dram_tensor("wWh", [A], F16, kind="ExternalInput")
    sWT_d = nc.dram_tensor("sWT", [S, A], F16, kind="ExternalInput")
    WihT_d = nc.dram_tensor("WihT", [A + DX, 3 * S], F16, kind="ExternalInput")
    WhhT_d = nc.dram_tensor("WhhT", [S, 3 * S], F16, kind="ExternalInput")
    fcWT_d = nc.dram_tensor("fcWT", [S, C], F16, kind="ExternalInput")
    ypT_d = nc.dram_tensor("ypT", [steps, A, bl], F16, kind="ExternalInput")
    sbv_d = nc.dram_tensor("sbv", [A], F32, kind="ExternalInput")
    xbv_d = nc.dram_tensor("xbv", [A], F32, kind="ExternalInput")
    bih_d = nc.dram_tensor("bihv", [3 * S], F32, kind="ExternalInput")
    bhh_d = nc.dram_tensor("bhhv", [3 * S], F32, kind="ExternalInput")
    fcb_d = nc.dram_tensor("fcbv", [C], F32, kind="ExternalInput")
    out_d = nc.dram_tensor("out", [bl, steps, C], F32, kind="ExternalOutput")

    def bcast_rows(ap, p):
        # replicate a free-dim-only AP across p partitions (step-0 partition dim)
        return bass.AP(tensor=ap.tensor, offset=ap.offset, ap=[[0, p]] + list(ap.ap))

    with tile.TileContext(nc) as tc:
        with tc.tile_pool(name="const", bufs=1) as CP, \
             tc.tile_pool(name="state", bufs=1) as SP, \
             tc.tile_pool(name="dram", bufs=1, space="DRAM") as DP:
            # ---- replicated parameters to SBUF ----
            xWT_sb = CP.tile([128, 4, A], F16)
            nc.sync.dma_start(xWT_sb, xWT_d[:].rearrange("(dc p) a -> p dc a", p=128))
            wW_sb = CP.tile([128, 4], F16)
            nc.sync.dma_start(wW_sb, wW_d[:].rearrange("(ac p) -> p ac", p=128))
            sWT_sb = CP.tile([128, 4, A], F16)
            nc.sync.dma_start(sWT_sb, sWT_d[:].rearrange("(sc p) a -> p sc a", p=128))
            WihT_sb = CP.tile([128, 8, 3 * S], F16)
            nc.sync.dma_start(WihT_sb, WihT_d[:].rearrange("(ic p) o -> p ic o", p=128))
            WhhT_sb = CP.tile([128, 4, 3 * S], F16)
            nc.sync.dma_start(WhhT_sb, WhhT_d[:].rearrange("(sc p) o -> p sc o", p=128))
            fcWT_sb = CP.tile([128, 4, C], F16)
            nc.sync.dma_start(fcWT_sb, fcWT_d[:].rearrange("(sc p) c -> p sc c", p=128))
            ypT_sb = CP.tile([128, steps, 4, bl], F16)
            nc.sync.dma_start(ypT_sb, ypT_d[:].rearrange("l (ac p) b -> p l ac b", p=128))
            sbv_sb = CP.tile([128, 4], F32)
            nc.sync.dma_start(sbv_sb, sbv_d[:].rearrange("(ac p) -> p ac", p=128))
            xbv_sb = CP.tile([128, 4], F32)
            nc.sync.dma_start(xbv_sb, xbv_d[:].rearrange("(ac p) -> p ac", p=128))
            bihrep = CP.tile([bl, 3 * S], F32)
            nc.gpsimd.dma_start(out=bihrep, in_=bcast_rows(bih_d[:], bl))
            bhhrep = CP.tile([bl, 3 * S], F32)
            nc.gpsimd.dma_start(out=bhhrep, in_=bcast_rows(bhh_d[:], bl))
            fcbrep = CP.tile([bl, C], F32)
            nc.gpsimd.dma_start(out=fcbrep, in_=bcast_rows(fcb_d[:], bl))
            ident = CP.tile([32, 32], F32)
            make_identity(nc, ident)

            # ---- persistent state ----
            h_sb = SP.tile([bl, S], F32)
            hT_sb = SP.tile([128, 4, bl], F16)
            sPT_sb = SP.tile([128, 4, bl], F32)
            alphaT_sb = SP.tile([128, TK, bl], F16)
            ctx_sb = SP.tile([bl, DX], F32)
            ctxT_sb = SP.tile([128, 4, bl], F16)
            u_sb = SP.tile([bl, 3 * S], F32)
            v_sb = SP.tile([bl, 3 * S], F32)
            r_sb = SP.tile([bl, S], F32)
            z_sb = SP.tile([bl, S], F32)
            n_sb = SP.tile([bl, S], F32)
            ta_sb = SP.tile([bl, S], F32)
            tb_sb = SP.tile([bl, S], F32)
            tg_sb = SP.tile([bl, S], F32)
            outbuf = SP.tile([bl, steps, C], F32)

            xh_d = [DP.tile([td, DX], F16, name=f"xh{b}") for b in range(bl)]
            xpj_d = [DP.tile([128, 4, td], F16, name=f"xpj{b}") for b in range(bl)]

            nc.vector.memset(h_sb, 0.0)
            nc.vector.memset(hT_sb, 0.0)
            # step-0 sProjT = sb (h0 = 0), broadcast across b
            nc.vector.memset(sPT_sb, 0.0)
            for ac in range(4):
                nc.vector.tensor_scalar_add(out=sPT_sb[:, ac, :],
                                            in0=sPT_sb[:, ac, :],
                                            scalar1=sbv_sb[:, ac:ac + 1])

            def tr(ps, in_ap):
                p = in_ap.shape[0]
                nc.tensor.transpose(ps, in_ap, ident[:p, :p])

            # ---- precompute: cast x to fp16; xpjT = fp16(x @ xW.T + xb) ----
            with tc.tile_pool(name="pre", bufs=2) as PRE, \
                 tc.tile_pool(name="prep", bufs=4, space="PSUM") as PPS, \
                 tc.tile_pool(name="xt", bufs=6) as XT:
                xr = x_in[:].rearrange("b (tq j p) d -> b tq j p d", j=4, p=128)
                for b in range(bl):
                    xh_w = xh_d[b][:].rearrange("(tq j p) d -> tq p j d", j=4, p=128)
                    for tq in range(TQ):
                        cin = PRE.tile([128, 4, DX], F32, tag="cin")
                        nc.sync.dma_start(cin, xr[b, tq].rearrange("j p d -> p j d"))
                        cot = PRE.tile([128, 4, DX], F16, tag="cot")
                        nc.vector.tensor_copy(out=cot, in_=cin)
                        nc.sync.dma_start(xh_w[tq], cot)
                    xts = []
                    for dc in range(4):
                        xt_t = XT.tile([128, td], F16, tag="xt")
                        nc.sync.dma_start_transpose(xt_t, xh_d[b][:, dc * 128:(dc + 1) * 128])
                        xts.append(xt_t)
                    for ac in range(4):
                        for tcc in range(TQ):
                            ps = PPS.tile([128, 512], F32, tag="pps")
                            for dc in range(4):
                                nc.tensor.matmul(
                                    ps,
                                    xWT_sb[:, dc, ac * 128:(ac + 1) * 128],
                                    xts[dc][:, tcc * 512:(tcc + 1) * 512],
                                    start=(dc == 0), stop=(dc == 3),
                                )
                            po = PRE.tile([128, 512], F16, tag="po")
                            nc.scalar.activation(out=po, in_=ps, func=AF.Identity,
                                                 bias=xbv_sb[:, ac:ac + 1])
                            nc.sync.dma_start(xpj_d[b][:, ac, tcc * 512:(tcc + 1) * 512], po)

            # ---- main sequential loop ----
            with tc.tile_pool(name="xp", bufs=3) as XP, \
                 tc.tile_pool(name="th", bufs=4) as TH, \
                 tc.tile_pool(name="xs", bufs=stream_bufs) as XS, \
                 tc.tile_pool(name="row", bufs=4) as ROW, \
                 tc.tile_pool(name="psrow", bufs=sc_bufs, space="PSUM") as PS_ROW, \
                 tc.tile_pool(name="pstr", bufs=2, space="PSUM") as PS_TR, \
                 tc.tile_pool(name="psg", bufs=2, space="PSUM") as PS_G, \
                 tc.tile_pool(name="psfc", bufs=1, space="PSUM") as PS_FC:
                GB = min(8, bl)  # batch rows per softmax group
                NG = bl // GB
                for i in range(steps):
                    for g in range(NG):
                        bs = range(g * GB, (g + 1) * GB)
                        # -- scores for this group --
                        sg = ROW.tile([GB, td], F32, tag="sg")
                        for b in bs:
                            srow = ROW.tile([1, td], F32, tag="srow")
                            scps = [PS_ROW.tile([1, 512], F32, tag="row", name=f"scps{q}")
                                    for q in range(TQ)]
                            xpt = XP.tile([128, 4, td], F16, tag="xp")
                            nc.sync.dma_start(xpt, xpj_d[b][:])
                            for ac in range(4):
                                tht = TH.tile([128, td], F16, tag="th")
                                nc.scalar.activation(out=tht, in_=xpt[:, ac, :], func=AF.Tanh,
                                                     bias=sPT_sb[:, ac, b:b + 1])
                                for tcc in range(TQ):
                                    nc.tensor.matmul(
                                        scps[tcc], wW_sb[:, ac:ac + 1],
                                        tht[:, tcc * 512:(tcc + 1) * 512],
                                        start=(ac == 0), stop=(ac == 3),
                                    )
                            for tcc in range(TQ):
                                nc.vector.tensor_copy(out=srow[:, tcc * 512:(tcc + 1) * 512],
                                                      in_=scps[tcc])
                            nc.sync.dma_start(sg[b - g * GB:b - g * GB + 1, :], srow)
                        # -- group softmax + alphaT columns --
                        ag = ROW.tile([GB, td], F32, tag="ag")
                        gsum = ROW.tile([GB, 1], F32, tag="gsum")
                        nc.scalar.activation(out=ag, in_=sg, func=AF.Exp, accum_out=gsum)
                        grinv = ROW.tile([GB, 1], F32, tag="grinv")
                        nc.vector.reciprocal(grinv, gsum)
                        nc.vector.tensor_scalar_mul(ag, ag, grinv)
                        for tk in range(TK):
                            tp = PS_TR.tile([128, GB], F32, tag="tr")
                            tr(tp, ag[:, tk * 128:(tk + 1) * 128])
                            nc.vector.tensor_copy(out=alphaT_sb[:, tk, g * GB:(g + 1) * GB],
                                                  in_=tp)
                        # -- context for this group --
                        for b in bs:
                            cps = PS_ROW.tile([1, DX], F32, tag="row")
                            xs_r = xh_d[b][:].rearrange("(pp j p) d -> pp p j d", j=2, p=128)
                            for pp in range(PP):
                                xst = XS.tile([128, 2, DX], F16, tag="xs")
                                nc.sync.dma_start(xst, xs_r[pp])
                                for j in range(2):
                                    tk = pp * 2 + j
                                    nc.tensor.matmul(cps, alphaT_sb[:, tk, b:b + 1],
                                                     xst[:, j, :],
                                                     start=(tk == 0), stop=(tk == TK - 1))
                            crow = ROW.tile([1, DX], F32, tag="crow")
                            nc.vector.tensor_copy(out=crow, in_=cps)
                            nc.sync.dma_start(ctx_sb[b:b + 1, :], crow)
                    for sc in range(4):
                        tp = PS_TR.tile([128, bl], F32, tag="tr")
                        tr(tp, ctx_sb[:, sc * 128:(sc + 1) * 128])
                        nc.vector.tensor_copy(out=ctxT_sb[:, sc, :], in_=tp)
                    # -- GRU --
                    for oc in range(3):
                        gip = PS_G.tile([bl, 512], F32, tag="g")
                        for ic in range(8):
                            lhsT = ypT_sb[:, i, ic, :] if ic < 4 else ctxT_sb[:, ic - 4, :]
                            nc.tensor.matmul(gip, lhsT,
                                             WihT_sb[:, ic, oc * 512:(oc + 1) * 512],
                                             start=(ic == 0), stop=(ic == 7))
                        ghp = PS_G.tile([bl, 512], F32, tag="g")
                        for sc in range(4):
                            nc.tensor.matmul(ghp, hT_sb[:, sc, :],
                                             WhhT_sb[:, sc, oc * 512:(oc + 1) * 512],
                                             start=(sc == 0), stop=(sc == 3))
                        nc.vector.tensor_tensor(v_sb[:, oc * 512:(oc + 1) * 512], gip,
                                                bihrep[:, oc * 512:(oc + 1) * 512], OP.add)
                        nc.vector.tensor_tensor(u_sb[:, oc * 512:(oc + 1) * 512], ghp,
                                                bhhrep[:, oc * 512:(oc + 1) * 512], OP.add)
                    nc.vector.tensor_tensor(ta_sb, v_sb[:, 0:512], u_sb[:, 0:512], OP.add)
                    nc.scalar.activation(out=r_sb, in_=ta_sb, func=AF.Sigmoid)
                    nc.vector.tensor_tensor(tb_sb, v_sb[:, 512:1024], u_sb[:, 512:1024], OP.add)
                    nc.scalar.activation(out=z_sb, in_=tb_sb, func=AF.Sigmoid)
                    nc.vector.tensor_tensor(tg_sb, r_sb, u_sb[:, 1024:1536], OP.mult)
                    nc.vector.tensor_tensor(tg_sb, v_sb[:, 1024:1536], tg_sb, OP.add)
                    nc.scalar.activation(out=n_sb, in_=tg_sb, func=AF.Tanh)
                    nc.vector.tensor_tensor(ta_sb, h_sb, n_sb, OP.subtract)
                    nc.vector.tensor_tensor(tb_sb, z_sb, ta_sb, OP.mult)
                    nc.vector.tensor_tensor(h_sb, n_sb, tb_sb, OP.add)
                    for sc in range(4):
                        tp = PS_TR.tile([128, bl], F32, tag="tr")
                        tr(tp, h_sb[:, sc * 128:(sc + 1) * 128])
                        nc.vector.tensor_copy(out=hT_sb[:, sc, :], in_=tp)
                    if i + 1 < steps:
                        for ac in range(4):
                            sp = PS_TR.tile([128, bl], F32, tag="tr")
                            for sc in range(4):
                                nc.tensor.matmul(sp,
                                                 sWT_sb[:, sc, ac * 128:(ac + 1) * 128],
                                                 hT_sb[:, sc, :],
                                                 start=(sc == 0), stop=(sc == 3))
                            nc.scalar.activation(out=sPT_sb[:, ac, :], in_=sp,
                                                 func=AF.Identity,
                                                 bias=sbv_sb[:, ac:ac + 1])
                    fp = PS_FC.tile([bl, C], F32, tag="fc")
                    for sc in range(4):
                        nc.tensor.matmul(fp, hT_sb[:, sc, :], fcWT_sb[:, sc, :],
                                         start=(sc == 0), stop=(sc == 3))
                    nc.vector.tensor_tensor(outbuf[:, i, :], fp, fcbrep, OP.add)
                nc.sync.dma_start(out_d[:], outbuf[:])

    nc.finalize()
    return nc


def host_prep(inputs, bl=BL, td=T, steps=L, ncores=NCORES):
    """Split the full inputs into per-core in_maps."""
    x = np.ascontiguousarray(np.asarray(inputs["x"], dtype=np.float32))
    targets = np.asarray(inputs["targets"], dtype=np.int32)
    emb = np.asarray(inputs["emb"], dtype=np.float32)
    num_classes = np.asarray(inputs["fcW"]).shape[0]
    nb = bl * ncores
    y_prev = np.concatenate(
        [np.full((nb, 1), num_classes, np.int32), targets[:nb, :steps - 1]], axis=1)
    yp = emb[y_prev]  # [nb, steps, A]

    shared = {
        "xWT": np.ascontiguousarray(np.asarray(inputs["xW"], np.float32).T).astype(np.float16),
        "wWh": np.asarray(inputs["wW"], np.float32).astype(np.float16),
        "sWT": np.ascontiguousarray(np.asarray(inputs["sW"], np.float32).T).astype(np.float16),
        "WihT": np.ascontiguousarray(np.asarray(inputs["W_ih"], np.float32).T).astype(np.float16),
        "WhhT": np.ascontiguousarray(np.asarray(inputs["W_hh"], np.float32).T).astype(np.float16),
        "fcWT": np.ascontiguousarray(np.asarray(inputs["fcW"], np.float32).T).astype(np.float16),
        "sbv": np.asarray(inputs["sb"], np.float32),
        "xbv": np.asarray(inputs["xb"], np.float32),
        "bihv": np.asarray(inputs["b_ih"], np.float32),
        "bhhv": np.asarray(inputs["b_hh"], np.float32),
        "fcbv": np.asarray(inputs["fcb"], np.float32),
    }
    in_maps = []
    for c in range(ncores):
        sl = slice(c * bl, (c + 1) * bl)
        m = dict(shared)
        m["xin"] = np.ascontiguousarray(x[sl, :td, :])
        m["ypT"] = np.ascontiguousarray(np.transpose(yp[sl], (1, 2, 0))).astype(np.float16)
        in_maps.append(m)
    return in_maps


_CACHE = {}
LAST_RESULTS = None


def kernel(**inputs):
    global LAST_RESULTS
    from concourse.bass_utils import run_bass_kernel_spmd
    if "nc" not in _CACHE:
        _CACHE["nc"] = build()
    nc = _CACHE["nc"]
    in_maps = host_prep(inputs)
    trace = bool(os.environ.get("ATH_TRACE"))
    res = run_bass_kernel_spmd(nc, in_maps, core_ids=list(range(NCORES)), trace=trace)
    LAST_RESULTS = res
    out = np.concatenate([res.results[c]["out"] for c in range(NCORES)], axis=0)
    return np.ascontiguousarray(out.astype(np.float32))
